# revision 1
# baseline (speedup 1.0000x reference)
"""Trainium2 Bass kernel for nn_HammingL2 (pairwise Hamming-weighted L2 loss).

Math: per-LUT loss = sum_{i<j} W[i,j](v_i-v_j)^2 = d.(v*v) - v^T W v with
d = rowsum(W).  Summed over all LUTs this equals  sum_ij M_ij G_ij  where
G = V^T V  (Gram over all LUTs, [256,256]) and  M = diag(d) - W.

Strategy: data-parallel over 8 NeuronCores.  Each core streams its
[8192, 256] shard of `luts` from HBM (one big SBUF buffer, 10 block DMAs
on the two HWDGE rings) and accumulates the shard Gram G_c = V_c^T V_c on
the tensor engine (128 accumulating f32r matmuls into two [128,256] PSUM
tiles).  The M = diag(d)-W contraction runs ON DEVICE: M tiles are DMA'd
in at the tail of the lut stream (hidden behind the matmul drain) and two
vector tensor_tensor_reduce ops produce per-partition partial sums
[128,2]; only 1 KiB leaves the core.  Host sums 8*256 floats.

A short burst of dummy matmuls on scratch SBUF warms the PE (HAM clock
gate) during the DMA fill latency so the real matmul stream runs at
2.4 GHz from the first chunk.

The kernel is DMA-bound: ~8.6 MiB/core of f32 reads at ~358 GB/s/core.
"""

import numpy as np

N_CORES = 8
NUM_LUTS = 65536
L = 256               # LUT_SIZE
SHARD = NUM_LUTS // N_CORES   # 8192 LUTs per core
P = 128               # partitions
CHUNKS = SHARD // P   # 64 matmul chunks per core

# DMA block sizes in chunks (1 chunk = 128 LUT rows = [128, 256] f32 = 128 KiB).
# Within a block of q chunks, partition p holds q CONSECUTIVE shard rows
# (r0 + p*q + c) so each partition's DMA run is q KiB contiguous.  Blocks
# alternate between the two HWDGE rings; this layout measured ~354 GB/s
# aggregate.  Tapered tail so the PE drains right behind the last byte.
BLOCK_SIZES = [4] * 15 + [2, 1, 1]
assert sum(BLOCK_SIZES) == CHUNKS

N_WARMUP = 14         # dummy bf16 N=256 matmuls to warm the PE clock gate

# Mode string: comma-joined flags.
#   warm   - bf16 PE-warmup dummies
#   dev    - on-device M contraction (tiny output); default: host epilogue
#   swdge  - load M tiles via gpsimd SWDGE (default: tail of HWDGE rings)
#   f16    - cast Gram to fp16 in the PSUM->SBUF copies; fp16 output DMA
#   1bank  - single PSUM bank for both Gram halves, single copy
# "f32r" = no flags: stream blocks on 2 HWDGE rings -> 128 f32r matmuls ->
# 2 PSUM->SBUF copies -> 256 KiB Gram out, host M contraction.
# "tail2" = same, but the two PSUM->SBUF copies run in parallel (DVE + ACT)
# and each Gram half ships on its own HWDGE ring, overlapping the two HBM
# write receipts.  Trace-verified ~0.45us faster tail than "f32r" with no
# semaphore-teardown perturbation.  Adding "f16" casts the Gram to fp16 in
# the copies (no overflow risk: |G| <= ~1e4 << 65504; loss rel err ~1e-6)
# and halves the output transfer: tail measured 2.34us vs 2.62us.  Every
# other explored variant (PE warmup, on-device M contraction, big-block
# DMA, SWDGE M loads, raw-bass teardown) measured slower or unsafe -- the
# kernel sits at the structural floor: ~6.6us fixed engine preamble +
# ~24us DMA-roofline stream + ~2us DMA completion latency + ~2.8us fixed
# semaphore-teardown chain.
MODE = "tail2,f16"

_CACHE = {}


def _seed_ntff_hook():
    """Make `antenv.axon_hooks` importable so run_bass_kernel_spmd(trace=True)
    can capture NTFF profiles under axon.  No-op if already present."""
    import sys
    import types

    try:
        import antenv.axon_hooks  # noqa: F401
        return
    except Exception:
        pass
    mod = types.ModuleType("antenv.axon_hooks")
    mod._hook = None

    def set_axon_ntff_profile_hook(h):
        mod._hook = h

    def get_axon_ntff_profile_hook():
        if mod._hook is None:
            try:
                from trn_agent_boot.trn_boot import _ntff_profile_via_ctypes

                mod._hook = _ntff_profile_via_ctypes("/opt/axon/libaxon_pjrt.so")
            except Exception:
                return None
        return mod._hook

    mod.set_axon_ntff_profile_hook = set_axon_ntff_profile_hook
    mod.get_axon_ntff_profile_hook = get_axon_ntff_profile_hook
    sys.modules["antenv.axon_hooks"] = mod


def _build_raw():
    """Raw-bass version of the f32r/tail2 kernel: 7 manual semaphores
    instead of TileContext's ~290, eliminating most of the serialized
    semaphore-teardown chain at program end and the Tile entry overhead.

    Engines: Sync issues even blocks + out half 0; Scalar issues odd
    blocks, ACT-copies Gram half 1, issues out half 1; Tensor runs the
    128 accumulating matmuls gated per-block on the per-ring DMA
    semaphores (HWDGE completes FIFO per ring); Vector copies half 0.
    """
    import concourse.mybir as mybir
    from concourse import bacc

    f32 = mybir.dt.float32
    f32r = mybir.dt.float32r
    nc = bacc.Bacc("TRN2", target_bir_lowering=False, debug=False, num_devices=N_CORES)
    v = nc.dram_tensor("v", [SHARD, L], f32r, kind="ExternalInput").ap()
    out = nc.dram_tensor("out", [P, 2, L], f32, kind="ExternalOutput").ap()

    # (bi, blk, chunk0, row0, ring, per-ring index)
    blocks = []
    c0 = 0
    r0 = 0
    na = nb = 0
    for bi, blk in enumerate(BLOCK_SIZES):
        ring = bi % 2
        if ring == 0:
            na += 1
            idx = na
        else:
            nb += 1
            idx = nb
        blocks.append((bi, blk, c0, r0, ring, idx))
        c0 += blk
        r0 += P * blk

    with (
        nc.sbuf_tensor([P, CHUNKS, L], f32r) as vt,
        nc.sbuf_tensor([P, 2, L], f32) as o_tile,
        nc.psum_tensor([P, L], f32) as g0,
        nc.psum_tensor([P, L], f32) as g1,
        nc.semaphore() as sem_a,
        nc.semaphore() as sem_b,
        nc.semaphore() as mm0_sem,
        nc.semaphore() as mm1_sem,
        nc.semaphore() as cp0_sem,
        nc.semaphore() as cp1_sem,
        nc.semaphore() as od_sem,
        nc.Block() as block,
    ):

        @block.sync
        def _(sync):
            for bi, blk, c0, r0, ring, idx in blocks:
                if ring == 0:
                    src = v[r0 : r0 + P * blk].rearrange("(p q) j -> p q j", q=blk)
                    sync.dma_start(vt[:, c0 : c0 + blk, :], src).then_inc(sem_a, 16)
            # out half 0 after the DVE copy's write has landed
            sync.wait_ge(cp0_sem, 1)
            sync.dma_start(out[:, 0, :], o_tile[:, 0, :]).then_inc(od_sem, 16)

        @block.scalar
        def _(scalar):
            for bi, blk, c0, r0, ring, idx in blocks:
                if ring == 1:
                    src = v[r0 : r0 + P * blk].rearrange("(p q) j -> p q j", q=blk)
                    scalar.dma_start(vt[:, c0 : c0 + blk, :], src).then_inc(sem_b, 16)
            scalar.wait_ge(mm1_sem, 1)
            scalar.copy(o_tile[:, 1, :], g1[:]).then_inc(cp1_sem, 1)
            # self-wait: ensure the ACT write landed before HWDGE reads it
            scalar.wait_ge(cp1_sem, 1)
            scalar.dma_start(out[:, 1, :], o_tile[:, 1, :]).then_inc(od_sem, 16)

        @block.tensor
        def _(tensor):
            k = 0
            for bi, blk, c0, r0, ring, idx in blocks:
                tensor.wait_ge(sem_a if ring == 0 else sem_b, 16 * idx)
                for c in range(c0, c0 + blk):
                    rhs = vt[:, c, :]
                    mm0 = tensor.matmul(
                        g0[:], vt[:, c, 0:P], rhs,
                        start=(k == 0), stop=(k == CHUNKS - 1),
                    )
                    mm1 = tensor.matmul(
                        g1[:], vt[:, c, P:L], rhs,
                        start=(k == 0), stop=(k == CHUNKS - 1),
                    )
                    if k == CHUNKS - 1:
                        mm0.then_inc(mm0_sem, 1)
                        mm1.then_inc(mm1_sem, 1)
                    k += 1

        @block.vector
        def _(vector):
            vector.wait_ge(mm0_sem, 1)
            vector.tensor_copy(o_tile[:, 0, :], g0[:]).then_inc(cp0_sem, 1)

        @block.gpsimd
        def _(gpsimd):
            # Sole end-of-program guard: wait for both output DMAs, then
            # reset DMA completion state and all kernel semaphores so the
            # NEFF can be re-executed (the profiler runs it more than once).
            gpsimd.wait_ge(od_sem, 32)
            sems = [sem_a, sem_b, mm0_sem, mm1_sem, cp0_sem, cp1_sem, od_sem]
            nums = sorted(s.num for s in sems)
            assert nums == list(range(nums[0], nums[0] + len(nums)))
            sem_range = range(nums[0], nums[-1] + 1)
            gpsimd.dma_reset(sem_range)
            gpsimd.sem_clear(sem_range)

    nc.compile()
    return nc


def _build_mode(mode):
    import concourse.mybir as mybir
    import concourse.tile as tile
    from concourse import bacc

    flags = set(mode.split(",")) if mode != "f32r" else set()
    warm = "warm" in flags
    dev = "dev" in flags
    swdge = "swdge" in flags
    f16 = "f16" in flags
    onebank = "1bank" in flags

    f32 = mybir.dt.float32
    f32r = mybir.dt.float32r
    bf16 = mybir.dt.bfloat16
    o_dt = mybir.dt.float16 if f16 else f32
    nc = bacc.Bacc("TRN2", target_bir_lowering=False, debug=False, num_devices=N_CORES)
    v = nc.dram_tensor("v", [SHARD, L], f32r, kind="ExternalInput").ap()
    if dev:
        m0 = nc.dram_tensor("m0", [P, L], f32, kind="ExternalInput").ap()
        m1 = nc.dram_tensor("m1", [P, L], f32, kind="ExternalInput").ap()
        out = nc.dram_tensor("out", [P, 2], f32, kind="ExternalOutput").ap()
    else:
        out = nc.dram_tensor("out", [P, 2, L], o_dt, kind="ExternalOutput").ap()

    max_q = max(BLOCK_SIZES)

    with tile.TileContext(nc) as tc:
        with (
            tc.tile_pool(name="vpool", bufs=len(BLOCK_SIZES)) as vpool,
            tc.tile_pool(name="mpool", bufs=1) as mpool,
            tc.tile_pool(name="psum", bufs=1, space="PSUM") as psum_pool,
            tc.tile_pool(name="opool", bufs=1) as opool,
        ):
            if onebank:
                gb = psum_pool.tile([P, 2, L], f32, tag="g", name="g")
                g_ps = [gb[:, 0, :], gb[:, 1, :]]
            else:
                g0 = psum_pool.tile([P, L], f32, tag="g0", name="g0")
                g1 = psum_pool.tile([P, L], f32, tag="g1", name="g1")
                g_ps = [g0[:], g1[:]]

            if warm:
                # PE warmup: cheap bf16 scratch matmuls (no data deps) run
                # during the DMA fill latency and flip the HAM clock gate to
                # 8/8 before the real stream starts.
                wt = mpool.tile([P, L], bf16, tag="w", name="wt")
                gw = psum_pool.tile([P, L], f32, tag="gw", name="gw")
                nc.vector.memset(wt[:], 0.0)
                for _ in range(N_WARMUP):
                    nc.tensor.matmul(gw[:], wt[:, 0:P], wt[:],
                                     start=True, stop=True)

            if dev:
                m0t = mpool.tile([P, L], f32, tag="m0", name="m0t")
                m1t = mpool.tile([P, L], f32, tag="m1", name="m1t")
                if swdge:
                    # M halves via the (otherwise idle) SWDGE ring.
                    nc.gpsimd.dma_start(m0t[:], m0)
                    nc.gpsimd.dma_start(m1t[:], m1)

            # Stream the shard, alternating blocks across the two HWDGE rings.
            vts = []
            r0 = 0
            for bi, blk in enumerate(BLOCK_SIZES):
                src = v[r0 : r0 + P * blk].rearrange("(p q) j -> p q j", q=blk)
                vt = vpool.tile([P, max_q, L], f32r, tag="v", name="vt")
                eng = nc.sync if bi % 2 == 0 else nc.scalar
                eng.dma_start(vt[:, :blk, :], src)
                vts.append((vt, blk))
                r0 += P * blk
            if dev and not swdge:
                # M halves at the tail of each HWDGE ring: they land right
                # at stream end, hidden behind the final matmul drain.
                nc.sync.dma_start(m0t[:], m0)
                nc.scalar.dma_start(m1t[:], m1)

            # Gram accumulation: G rows [0:128] into g0, rows [128:256]
            # into g1 (separate PSUM banks - separate accumulation chains).
            k = 0
            for vt, blk in vts:
                for c in range(blk):
                    rhs = vt[:, c, :]
                    for h in range(2):
                        # With both halves in one PSUM bank, only the very
                        # first matmul clears the bank (start=True clears
                        # bank-wide has_written bits).
                        st = (k == 0 and h == 0) if onebank else (k == 0)
                        nc.tensor.matmul(
                            g_ps[h],
                            vt[:, c, h * P : (h + 1) * P],
                            rhs,
                            start=st,
                            stop=(k == CHUNKS - 1),
                        )
                    k += 1

            if dev:
                # Fused on-device M contraction per half:
                #   res[p, h] = sum_j G_h[p, j] * M_h[p, j]
                prod = opool.tile([P, L], f32, tag="prod", name="prod")
                res = opool.tile([P, 2], f32, tag="res", name="res")
                nc.vector.affine_mul_reduce(
                    out=prod[:], accum_out=res[:, 0:1], in0=g_ps[0], in1=m0t[:],
                    scale=1.0, bias=0.0,
                )
                nc.vector.affine_mul_reduce(
                    out=prod[:], accum_out=res[:, 1:2], in0=g_ps[1], in1=m1t[:],
                    scale=1.0, bias=0.0,
                )
                nc.sync.dma_start(out, res[:])
            else:
                o_tile = opool.tile([P, 2, L], o_dt, tag="o")
                if onebank:
                    nc.vector.tensor_copy(o_tile[:], gb[:])
                    nc.sync.dma_start(out, o_tile[:])
                elif "tail2" in flags:
                    # Parallel tail: the slower ACT copy takes g0 (whose stop
                    # matmul retires one MM earlier), the faster DVE copy
                    # takes g1; each half then ships on its own HWDGE ring so
                    # the two HBM write receipts overlap.
                    nc.scalar.copy(o_tile[:, 0, :], g_ps[0])
                    nc.vector.tensor_copy(o_tile[:, 1, :], g_ps[1])
                    nc.scalar.dma_start(out[:, 0, :], o_tile[:, 0, :])
                    nc.sync.dma_start(out[:, 1, :], o_tile[:, 1, :])
                elif "pcopy" in flags:
                    # copies on two engines in parallel
                    nc.vector.tensor_copy(o_tile[:, 0, :], g_ps[0])
                    nc.gpsimd.tensor_copy(o_tile[:, 1, :], g_ps[1])
                    nc.sync.dma_start(out, o_tile[:])
                else:
                    for h in range(2):
                        nc.vector.tensor_copy(o_tile[:, h, :], g_ps[h])
                    nc.sync.dma_start(out, o_tile[:])

    nc.compile()
    return nc


def _build(mode=None):
    mode = mode or MODE
    if mode in _CACHE:
        return _CACHE[mode]
    nc = _build_raw() if mode == "raw" else _build_mode(mode)
    _CACHE[mode] = nc
    return nc


def _m_tiles(W):
    """M = diag(rowsum(W)) - W split into row halves [128, 256] each."""
    Wd = np.asarray(W, dtype=np.float64)
    M = np.diag(Wd.sum(axis=1)) - Wd
    m0 = np.ascontiguousarray(M[:P, :], dtype=np.float32)
    m1 = np.ascontiguousarray(M[P:, :], dtype=np.float32)
    return m0, m1


def _run(luts, W, trace=False, mode=None, **trace_kwargs):
    """Shard, run on 8 cores, return (loss_scalar, BassKernelResults)."""
    _seed_ntff_hook()
    from concourse.bass_utils import run_bass_kernel_spmd

    mode = mode or MODE
    nc = _build(mode)

    luts = np.ascontiguousarray(np.asarray(luts, dtype=np.float32))
    W = np.asarray(W, dtype=np.float32)

    if "dev" in mode:
        m0, m1 = _m_tiles(W)
        in_maps = [
            {"v": luts[i * SHARD : (i + 1) * SHARD], "m0": m0, "m1": m1}
            for i in range(N_CORES)
        ]
        res = run_bass_kernel_spmd(
            nc, in_maps, core_ids=list(range(N_CORES)), trace=trace, **trace_kwargs
        )
        total = sum(r["out"].astype(np.float64).sum() for r in res.results)
        loss = np.asarray(total / NUM_LUTS, dtype=np.float32)
        return loss, res

    in_maps = [{"v": luts[i * SHARD : (i + 1) * SHARD]} for i in range(N_CORES)]
    res = run_bass_kernel_spmd(
        nc, in_maps, core_ids=list(range(N_CORES)), trace=trace, **trace_kwargs
    )
    Wd = W.astype(np.float64)
    M = np.diag(Wd.sum(axis=1)) - Wd
    G = np.zeros((L, L), dtype=np.float64)
    for r in res.results:
        g = r["out"].astype(np.float64)  # [128, 2, 256]
        G[:P] += g[:, 0, :]
        G[P:] += g[:, 1, :]
    loss = np.asarray((M * G).sum() / NUM_LUTS, dtype=np.float32)
    return loss, res


def kernel(luts, W, gamma=None, **_unused):
    loss, _ = _run(luts, W, trace=False)
    return loss


if __name__ == "__main__":
    rng = np.random.default_rng(0)
    luts = rng.standard_normal((NUM_LUTS, L), dtype=np.float32)
    W = rng.random((L, L), dtype=np.float32)
    W = (W + W.T) / 2
    np.fill_diagonal(W, 0.0)
    print(kernel(luts, W))



# revision 7
# speedup vs baseline: 1.5893x; 1.5893x over previous
"""Trainium2 Bass kernel for nn_HammingL2 (pairwise Hamming-weighted L2 loss).

Math: per-LUT loss = sum_{i<j} W[i,j](v_i-v_j)^2 = d.(v*v) - v^T W v with
d = rowsum(W).  Summed over all LUTs this equals  sum_ij M_ij G_ij  where
G = V^T V  (Gram over all LUTs, [256,256]) and  M = diag(d) - W.

Strategy: data-parallel over 8 NeuronCores.  Each core streams its
[8192, 256] shard of `luts` from HBM (one big SBUF buffer, 10 block DMAs
on the two HWDGE rings) and accumulates the shard Gram G_c = V_c^T V_c on
the tensor engine (128 accumulating f32r matmuls into two [128,256] PSUM
tiles).  The M = diag(d)-W contraction runs ON DEVICE: M tiles are DMA'd
in at the tail of the lut stream (hidden behind the matmul drain) and two
vector tensor_tensor_reduce ops produce per-partition partial sums
[128,2]; only 1 KiB leaves the core.  Host sums 8*256 floats.

A short burst of dummy matmuls on scratch SBUF warms the PE (HAM clock
gate) during the DMA fill latency so the real matmul stream runs at
2.4 GHz from the first chunk.

The kernel is DMA-bound: ~8.6 MiB/core of f32 reads at ~358 GB/s/core.
"""

import numpy as np

N_CORES = 8
NUM_LUTS = 65536
L = 256               # LUT_SIZE
SHARD = NUM_LUTS // N_CORES   # 8192 LUTs per core
P = 128               # partitions
CHUNKS = SHARD // P   # 64 matmul chunks per core

# DMA block sizes in chunks (1 chunk = 128 LUT rows = [128, 256] f32 = 128 KiB).
# Within a block of q chunks, partition p holds q CONSECUTIVE shard rows
# (r0 + p*q + c) so each partition's DMA run is q KiB contiguous.  Blocks
# alternate between the two HWDGE rings; this layout measured ~354 GB/s
# aggregate.  Tapered tail so the PE drains right behind the last byte.
BLOCK_SIZES = [4] * 15 + [2, 1, 1]
assert sum(BLOCK_SIZES) == CHUNKS

# fp8 blocks must be even-sized so DoubleRow chunk-pairs never span two
# tiles.  A chunk is [128, 256] fp8 = 32 KiB; per-partition contiguous run
# within a block of q chunks is q*256 B.
BLOCK_SIZES_FP8 = [4] * 15 + [2, 2]
assert sum(BLOCK_SIZES_FP8) == CHUNKS

N_WARMUP = 14         # dummy bf16 N=256 matmuls to warm the PE clock gate

# Mode string: comma-joined flags.
#   warm   - bf16 PE-warmup dummies
#   dev    - on-device M contraction (tiny output); default: host epilogue
#   swdge  - load M tiles via gpsimd SWDGE (default: tail of HWDGE rings)
#   f16    - cast Gram to fp16 in the PSUM->SBUF copies; fp16 output DMA
#   1bank  - single PSUM bank for both Gram halves, single copy
# "f32r" = no flags: stream blocks on 2 HWDGE rings -> 128 f32r matmuls ->
# 2 PSUM->SBUF copies -> 256 KiB Gram out, host M contraction.
# "tail2" = same, but the two PSUM->SBUF copies run in parallel (DVE + ACT)
# and each Gram half ships on its own HWDGE ring, overlapping the two HBM
# write receipts.  Trace-verified ~0.45us faster tail than "f32r" with no
# semaphore-teardown perturbation.  Adding "f16" casts the Gram to fp16 in
# the copies (no overflow risk: |G| <= ~1e4 << 65504; loss rel err ~1e-6)
# and halves the output transfer: tail measured 2.34us vs 2.62us.  Every
# other explored variant (PE warmup, on-device M contraction, big-block
# DMA, SWDGE M loads, raw-bass teardown) measured slower or unsafe -- the
# kernel sits at the structural floor: ~6.6us fixed engine preamble +
# ~24us DMA-roofline stream + ~2us DMA completion latency + ~2.8us fixed
# semaphore-teardown chain.
MODE = "tail2,f16"

_CACHE = {}


def _seed_ntff_hook():
    """Make `antenv.axon_hooks` importable so run_bass_kernel_spmd(trace=True)
    can capture NTFF profiles under axon.  No-op if already present."""
    import sys
    import types

    try:
        import antenv.axon_hooks  # noqa: F401
        return
    except Exception:
        pass
    mod = types.ModuleType("antenv.axon_hooks")
    mod._hook = None

    def set_axon_ntff_profile_hook(h):
        mod._hook = h

    def get_axon_ntff_profile_hook():
        if mod._hook is None:
            try:
                from trn_agent_boot.trn_boot import _ntff_profile_via_ctypes

                mod._hook = _ntff_profile_via_ctypes("/opt/axon/libaxon_pjrt.so")
            except Exception:
                return None
        return mod._hook

    mod.set_axon_ntff_profile_hook = set_axon_ntff_profile_hook
    mod.get_axon_ntff_profile_hook = get_axon_ntff_profile_hook
    sys.modules["antenv.axon_hooks"] = mod


def _build_raw():
    """Raw-bass version of the f32r/tail2 kernel: 7 manual semaphores
    instead of TileContext's ~290, eliminating most of the serialized
    semaphore-teardown chain at program end and the Tile entry overhead.

    Engines: Sync issues even blocks + out half 0; Scalar issues odd
    blocks, ACT-copies Gram half 1, issues out half 1; Tensor runs the
    128 accumulating matmuls gated per-block on the per-ring DMA
    semaphores (HWDGE completes FIFO per ring); Vector copies half 0.
    """
    import concourse.mybir as mybir
    from concourse import bacc

    f32 = mybir.dt.float32
    f32r = mybir.dt.float32r
    nc = bacc.Bacc("TRN2", target_bir_lowering=False, debug=False, num_devices=N_CORES)
    v = nc.dram_tensor("v", [SHARD, L], f32r, kind="ExternalInput").ap()
    out = nc.dram_tensor("out", [P, 2, L], f32, kind="ExternalOutput").ap()

    # (bi, blk, chunk0, row0, ring, per-ring index)
    blocks = []
    c0 = 0
    r0 = 0
    na = nb = 0
    for bi, blk in enumerate(BLOCK_SIZES):
        ring = bi % 2
        if ring == 0:
            na += 1
            idx = na
        else:
            nb += 1
            idx = nb
        blocks.append((bi, blk, c0, r0, ring, idx))
        c0 += blk
        r0 += P * blk

    with (
        nc.sbuf_tensor([P, CHUNKS, L], f32r) as vt,
        nc.sbuf_tensor([P, 2, L], f32) as o_tile,
        nc.psum_tensor([P, L], f32) as g0,
        nc.psum_tensor([P, L], f32) as g1,
        nc.semaphore() as sem_a,
        nc.semaphore() as sem_b,
        nc.semaphore() as mm0_sem,
        nc.semaphore() as mm1_sem,
        nc.semaphore() as cp0_sem,
        nc.semaphore() as cp1_sem,
        nc.semaphore() as od_sem,
        nc.Block() as block,
    ):

        @block.sync
        def _(sync):
            for bi, blk, c0, r0, ring, idx in blocks:
                if ring == 0:
                    src = v[r0 : r0 + P * blk].rearrange("(p q) j -> p q j", q=blk)
                    sync.dma_start(vt[:, c0 : c0 + blk, :], src).then_inc(sem_a, 16)
            # out half 0 after the DVE copy's write has landed
            sync.wait_ge(cp0_sem, 1)
            sync.dma_start(out[:, 0, :], o_tile[:, 0, :]).then_inc(od_sem, 16)

        @block.scalar
        def _(scalar):
            for bi, blk, c0, r0, ring, idx in blocks:
                if ring == 1:
                    src = v[r0 : r0 + P * blk].rearrange("(p q) j -> p q j", q=blk)
                    scalar.dma_start(vt[:, c0 : c0 + blk, :], src).then_inc(sem_b, 16)
            scalar.wait_ge(mm1_sem, 1)
            scalar.copy(o_tile[:, 1, :], g1[:]).then_inc(cp1_sem, 1)
            # self-wait: ensure the ACT write landed before HWDGE reads it
            scalar.wait_ge(cp1_sem, 1)
            scalar.dma_start(out[:, 1, :], o_tile[:, 1, :]).then_inc(od_sem, 16)

        @block.tensor
        def _(tensor):
            k = 0
            for bi, blk, c0, r0, ring, idx in blocks:
                tensor.wait_ge(sem_a if ring == 0 else sem_b, 16 * idx)
                for c in range(c0, c0 + blk):
                    rhs = vt[:, c, :]
                    mm0 = tensor.matmul(
                        g0[:], vt[:, c, 0:P], rhs,
                        start=(k == 0), stop=(k == CHUNKS - 1),
                    )
                    mm1 = tensor.matmul(
                        g1[:], vt[:, c, P:L], rhs,
                        start=(k == 0), stop=(k == CHUNKS - 1),
                    )
                    if k == CHUNKS - 1:
                        mm0.then_inc(mm0_sem, 1)
                        mm1.then_inc(mm1_sem, 1)
                    k += 1

        @block.vector
        def _(vector):
            vector.wait_ge(mm0_sem, 1)
            vector.tensor_copy(o_tile[:, 0, :], g0[:]).then_inc(cp0_sem, 1)

        @block.gpsimd
        def _(gpsimd):
            # Sole end-of-program guard: wait for both output DMAs, then
            # reset DMA completion state and all kernel semaphores so the
            # NEFF can be re-executed (the profiler runs it more than once).
            gpsimd.wait_ge(od_sem, 32)
            sems = [sem_a, sem_b, mm0_sem, mm1_sem, cp0_sem, cp1_sem, od_sem]
            nums = sorted(s.num for s in sems)
            assert nums == list(range(nums[0], nums[0] + len(nums)))
            sem_range = range(nums[0], nums[-1] + 1)
            gpsimd.dma_reset(sem_range)
            gpsimd.sem_clear(sem_range)

    nc.compile()
    return nc


def _build_mode(mode):
    import concourse.mybir as mybir
    import concourse.tile as tile
    from concourse import bacc

    flags = set(mode.split(",")) if mode != "f32r" else set()
    warm = "warm" in flags
    dev = "dev" in flags
    swdge = "swdge" in flags
    f16 = "f16" in flags
    onebank = "1bank" in flags
    fp8 = "fp8" in flags
    use_bf16 = "bf16" in flags

    f32 = mybir.dt.float32
    f32r = mybir.dt.float32r
    bf16 = mybir.dt.bfloat16
    v_dt = mybir.dt.float8e4 if fp8 else (bf16 if use_bf16 else f32r)
    block_sizes = BLOCK_SIZES_FP8 if fp8 else BLOCK_SIZES
    o_dt = mybir.dt.float16 if f16 else f32
    nc = bacc.Bacc("TRN2", target_bir_lowering=False, debug=False, num_devices=N_CORES)
    v = nc.dram_tensor("v", [SHARD, L], v_dt, kind="ExternalInput").ap()
    if dev:
        m0 = nc.dram_tensor("m0", [P, L], f32, kind="ExternalInput").ap()
        m1 = nc.dram_tensor("m1", [P, L], f32, kind="ExternalInput").ap()
        out = nc.dram_tensor("out", [P, 2], f32, kind="ExternalOutput").ap()
    else:
        out = nc.dram_tensor("out", [P, 2, L], o_dt, kind="ExternalOutput").ap()

    max_q = max(block_sizes)

    with tile.TileContext(nc) as tc:
        with (
            tc.tile_pool(name="vpool", bufs=len(block_sizes)) as vpool,
            tc.tile_pool(name="mpool", bufs=1) as mpool,
            tc.tile_pool(name="psum", bufs=1, space="PSUM") as psum_pool,
            tc.tile_pool(name="opool", bufs=1) as opool,
        ):
            if onebank:
                gb = psum_pool.tile([P, 2, L], f32, tag="g", name="g")
                g_ps = [gb[:, 0, :], gb[:, 1, :]]
            else:
                g0 = psum_pool.tile([P, L], f32, tag="g0", name="g0")
                g1 = psum_pool.tile([P, L], f32, tag="g1", name="g1")
                g_ps = [g0[:], g1[:]]

            if warm:
                # PE warmup: cheap bf16 scratch matmuls (no data deps) run
                # during the DMA fill latency and flip the HAM clock gate to
                # 8/8 before the real stream starts.
                wt = mpool.tile([P, L], bf16, tag="w", name="wt")
                gw = psum_pool.tile([P, L], f32, tag="gw", name="gw")
                nc.vector.memset(wt[:], 0.0)
                for _ in range(N_WARMUP):
                    nc.tensor.matmul(gw[:], wt[:, 0:P], wt[:],
                                     start=True, stop=True)

            if dev:
                m0t = mpool.tile([P, L], f32, tag="m0", name="m0t")
                m1t = mpool.tile([P, L], f32, tag="m1", name="m1t")
                if swdge:
                    # M halves via the (otherwise idle) SWDGE ring.
                    nc.gpsimd.dma_start(m0t[:], m0)
                    nc.gpsimd.dma_start(m1t[:], m1)

            # Stream the shard, alternating blocks across the two HWDGE rings.
            vts = []
            r0 = 0
            for bi, blk in enumerate(block_sizes):
                src = v[r0 : r0 + P * blk].rearrange("(p q) j -> p q j", q=blk)
                vt = vpool.tile([P, max_q, L], v_dt, tag="v", name="vt")
                eng = nc.sync if bi % 2 == 0 else nc.scalar
                eng.dma_start(vt[:, :blk, :], src)
                vts.append((vt, blk))
                r0 += P * blk
            if dev and not swdge:
                # M halves at the tail of each HWDGE ring: they land right
                # at stream end, hidden behind the final matmul drain.
                nc.sync.dma_start(m0t[:], m0)
                nc.scalar.dma_start(m1t[:], m1)

            # Gram accumulation: G rows [0:128] into g0, rows [128:256]
            # into g1 (separate PSUM banks - separate accumulation chains).
            if fp8:
                # DoubleRow perf mode: one matmul contracts TWO chunks
                # (effective K = 256) at 0.5 cycles/row.  lhsT [128, 2, 128],
                # rhs [128, 2, 256]; out = sum_i lhsT[:,i,:].T @ rhs[:,i,:].
                n_pairs = CHUNKS // 2
                k2 = 0
                for vt, blk in vts:
                    for c in range(0, blk, 2):
                        rhs = vt[:, c : c + 2, :]
                        for h in range(2):
                            nc.tensor.matmul(
                                g_ps[h],
                                vt[:, c : c + 2, h * P : (h + 1) * P],
                                rhs,
                                start=(k2 == 0),
                                stop=(k2 == n_pairs - 1),
                                perf_mode=mybir.MatmulPerfMode.DoubleRow,
                            )
                        k2 += 1
            else:
                k = 0
                for vt, blk in vts:
                    for c in range(blk):
                        rhs = vt[:, c, :]
                        for h in range(2):
                            # With both halves in one PSUM bank, only the very
                            # first matmul clears the bank (start=True clears
                            # bank-wide has_written bits).
                            st = (k == 0 and h == 0) if onebank else (k == 0)
                            nc.tensor.matmul(
                                g_ps[h],
                                vt[:, c, h * P : (h + 1) * P],
                                rhs,
                                start=st,
                                stop=(k == CHUNKS - 1),
                            )
                        k += 1

            if dev:
                # Fused on-device M contraction per half:
                #   res[p, h] = sum_j G_h[p, j] * M_h[p, j]
                prod = opool.tile([P, L], f32, tag="prod", name="prod")
                res = opool.tile([P, 2], f32, tag="res", name="res")
                nc.vector.affine_mul_reduce(
                    out=prod[:], accum_out=res[:, 0:1], in0=g_ps[0], in1=m0t[:],
                    scale=1.0, bias=0.0,
                )
                nc.vector.affine_mul_reduce(
                    out=prod[:], accum_out=res[:, 1:2], in0=g_ps[1], in1=m1t[:],
                    scale=1.0, bias=0.0,
                )
                nc.sync.dma_start(out, res[:])
            else:
                o_tile = opool.tile([P, 2, L], o_dt, tag="o")
                if onebank:
                    nc.vector.tensor_copy(o_tile[:], gb[:])
                    nc.sync.dma_start(out, o_tile[:])
                elif "tail2" in flags:
                    # Parallel tail: the slower ACT copy takes g0 (whose stop
                    # matmul retires one MM earlier), the faster DVE copy
                    # takes g1; each half then ships on its own HWDGE ring so
                    # the two HBM write receipts overlap.
                    nc.scalar.copy(o_tile[:, 0, :], g_ps[0])
                    nc.vector.tensor_copy(o_tile[:, 1, :], g_ps[1])
                    nc.scalar.dma_start(out[:, 0, :], o_tile[:, 0, :])
                    nc.sync.dma_start(out[:, 1, :], o_tile[:, 1, :])
                elif "pcopy" in flags:
                    # copies on two engines in parallel
                    nc.vector.tensor_copy(o_tile[:, 0, :], g_ps[0])
                    nc.gpsimd.tensor_copy(o_tile[:, 1, :], g_ps[1])
                    nc.sync.dma_start(out, o_tile[:])
                else:
                    for h in range(2):
                        nc.vector.tensor_copy(o_tile[:, h, :], g_ps[h])
                    nc.sync.dma_start(out, o_tile[:])

    nc.compile()
    return nc


def _build(mode=None):
    mode = mode or MODE
    if mode in _CACHE:
        return _CACHE[mode]
    nc = _build_raw() if mode == "raw" else _build_mode(mode)
    _CACHE[mode] = nc
    return nc


def _m_tiles(W):
    """M = diag(rowsum(W)) - W split into row halves [128, 256] each."""
    Wd = np.asarray(W, dtype=np.float64)
    M = np.diag(Wd.sum(axis=1)) - Wd
    m0 = np.ascontiguousarray(M[:P, :], dtype=np.float32)
    m1 = np.ascontiguousarray(M[P:, :], dtype=np.float32)
    return m0, m1


def _run(luts, W, trace=False, mode=None, **trace_kwargs):
    """Shard, run on 8 cores, return (loss_scalar, BassKernelResults)."""
    _seed_ntff_hook()
    from concourse.bass_utils import run_bass_kernel_spmd

    mode = mode or MODE
    nc = _build(mode)

    luts = np.ascontiguousarray(np.asarray(luts, dtype=np.float32))
    W = np.asarray(W, dtype=np.float32)

    if "fp8" in mode:
        # Quantize on host: TRN fp8e4 == ml_dtypes.float8_e4m3 (max 240).
        # randn data (|v| < ~5.5) never clips; loss rel err ~7e-4.
        import ml_dtypes

        luts = luts.astype(ml_dtypes.float8_e4m3)
    elif "bf16" in mode:
        import ml_dtypes

        luts = luts.astype(ml_dtypes.bfloat16)

    if "dev" in mode:
        m0, m1 = _m_tiles(W)
        in_maps = [
            {"v": luts[i * SHARD : (i + 1) * SHARD], "m0": m0, "m1": m1}
            for i in range(N_CORES)
        ]
        res = run_bass_kernel_spmd(
            nc, in_maps, core_ids=list(range(N_CORES)), trace=trace, **trace_kwargs
        )
        total = sum(r["out"].astype(np.float64).sum() for r in res.results)
        loss = np.asarray(total / NUM_LUTS, dtype=np.float32)
        return loss, res

    in_maps = [{"v": luts[i * SHARD : (i + 1) * SHARD]} for i in range(N_CORES)]
    res = run_bass_kernel_spmd(
        nc, in_maps, core_ids=list(range(N_CORES)), trace=trace, **trace_kwargs
    )
    Wd = W.astype(np.float64)
    M = np.diag(Wd.sum(axis=1)) - Wd
    G = np.zeros((L, L), dtype=np.float64)
    for r in res.results:
        g = r["out"].astype(np.float64)  # [128, 2, 256]
        G[:P] += g[:, 0, :]
        G[P:] += g[:, 1, :]
    loss = np.asarray((M * G).sum() / NUM_LUTS, dtype=np.float32)
    return loss, res


def kernel(luts, W, gamma=None, **_unused):
    loss, _ = _run(luts, W, trace=False)
    return loss


if __name__ == "__main__":
    rng = np.random.default_rng(0)
    luts = rng.standard_normal((NUM_LUTS, L), dtype=np.float32)
    W = rng.random((L, L), dtype=np.float32)
    W = (W + W.T) / 2
    np.fill_diagonal(W, 0.0)
    print(kernel(luts, W))

